# revision 1
# baseline (speedup 1.0000x reference)
"""LoRA linear layer on 8 Trainium2 NeuronCores.

Computes out = x @ (lora_B @ lora_A * 2).T + bias for
x [4, 2048, 4096], lora_A [16, 4096], lora_B [4096, 16], bias [4096].

Strategy: pure data parallel — shard x over batch*seq (8192 rows -> 1024
rows/core), replicate the tiny LoRA weights. Per core, exploit the rank-16
structure: y = x @ A^T (contract 4096), z = y @ B^T * 2 + bias (contract 16),
instead of materializing the 4096x4096 dense W. Memory-bound: 16 MiB in +
16 MiB out per core.

Per-core pipeline, super-tiles of 256 rows (4 per core):
  1. DMA two x row-tiles [128, 4096] into SBUF.
  2. PE-transpose x chunks [128,128] -> PSUM, batched 4 per [128,512] PSUM
     tile, one big copy each to the x^T SBUF buffer (fp32 has no
     DMA-transpose path; feature dim must sit on partitions for GEMM1).
  3. GEMM1: 32 accumulating matmuls, lhsT = A^T chunk [128,16] (pre-scaled
     by 2), rhs = x^T chunk [128,256] -> y^T [16,256] in PSUM.
  4. Bias trick: yT_ext = [y^T; ones] [17,256]; BB = [B^T; bias] [17,4096].
     GEMM2 per 128-row half: z chunk [128,512] = yT_ext[:,half] ^T-free
     matmul with BB chunk — bias is added by the matmul itself.
  5. Copy z PSUM -> SBUF (alternating ScalarE/VectorE), DMA out per row-tile.
"""

import sys

import numpy as np

if "/opt/trn_rl_repo" not in sys.path:
    sys.path.insert(0, "/opt/trn_rl_repo")

import concourse.bass as bass
import concourse.mybir as mybir
from concourse import bacc
from concourse.bass_utils import run_bass_kernel_spmd
from concourse.masks import make_identity
from concourse.tile import TileContext

N_CORES = 8
B, S, IN_F, OUT_F, R = 4, 2048, 4096, 4096, 16
ROWS = B * S // N_CORES  # 1024 rows per core
SCALING = 2.0  # alpha / r = 32 / 16
FP32 = mybir.dt.float32
P = 128
NK = IN_F // P  # 32 contraction chunks for GEMM1
SROWS = 256  # super-tile rows (GEMM1 moving free dim)
NS = ROWS // SROWS  # 4 super-tiles per core
HT = SROWS // P  # 2 row-tiles per super-tile
ZC = 512  # GEMM2 moving chunk (fp32 max free dim)
NJ = OUT_F // ZC  # 8 output chunks per row tile

_nc_cache = None


def build_nc() -> bass.Bass:
    nc = bacc.Bacc()
    x_d = nc.declare_dram_parameter("x", [ROWS, IN_F], FP32, isOutput=False)
    a_d = nc.declare_dram_parameter("lora_A", [R, IN_F], FP32, isOutput=False)
    b_d = nc.declare_dram_parameter("lora_B", [OUT_F, R], FP32, isOutput=False)
    bias_d = nc.declare_dram_parameter("bias", [1, OUT_F], FP32, isOutput=False)
    out_d = nc.declare_dram_parameter("out", [ROWS, OUT_F], FP32, isOutput=True)

    with TileContext(nc) as tc:
        with (
            tc.tile_pool(name="const", bufs=1) as const,
            tc.tile_pool(name="xin", bufs=3) as xin,
            tc.tile_pool(name="xtp", bufs=2) as xtp,
            tc.tile_pool(name="zrp", bufs=2) as zrp,
            tc.tile_pool(name="ytp", bufs=2) as ytp,
            tc.tile_pool(name="tpsum", bufs=4, space="PSUM") as tpsum,
            tc.tile_pool(name="ypsum", bufs=1, space="PSUM") as ypsum,
            tc.tile_pool(name="zpsum", bufs=3, space="PSUM") as zpsum,
        ):
            ident = const.tile([P, P], FP32)
            make_identity(nc, ident)

            # A^T chunks, pre-scaled: at_sb[:, 16k:16k+16] = 2 * A[:, 128k:128k+128]^T
            # Transposes batched 4-per-PSUM-tile so one ACT op copies+scales 4.
            # a_sb borrows a zrow slot (same free size, only needed at setup).
            a_sb = zrp.tile([R, IN_F], FP32, tag="z")
            nc.sync.dma_start(out=a_sb[:, :], in_=a_d[:, :])
            at_sb = const.tile([P, NK * R], FP32)
            for g in range(NK // 4):
                pt = tpsum.tile([P, ZC], FP32, tag="tp")
                for i in range(4):
                    k = 4 * g + i
                    nc.tensor.transpose(
                        pt[:, i * R : (i + 1) * R],
                        a_sb[:, k * P : (k + 1) * P],
                        ident[:R, :R],
                    )
                nc.scalar.mul(
                    out=at_sb[:, g * 4 * R : (g + 1) * 4 * R],
                    in_=pt[:, : 4 * R],
                    mul=SCALING,
                )

            # BB = [B^T; bias] with shape [17, 4096]
            b_sb = const.tile([P, NK * R], FP32)
            for k in range(NK):
                nc.sync.dma_start(
                    out=b_sb[:, k * R : (k + 1) * R], in_=b_d[k * P : (k + 1) * P, :]
                )
            bb = const.tile([R + 1, OUT_F], FP32)
            for g in range(NK // 4):
                pt = tpsum.tile([R, 4 * P], FP32, tag="tp")
                for i in range(4):
                    k = 4 * g + i
                    nc.tensor.transpose(
                        pt[:, i * P : (i + 1) * P],
                        b_sb[:, k * R : (k + 1) * R],
                        ident[:, :],
                    )
                nc.vector.tensor_copy(
                    out=bb[0:R, g * 4 * P : (g + 1) * 4 * P], in_=pt[:, :]
                )
            nc.sync.dma_start(out=bb[R : R + 1, :], in_=bias_d[:, :])

            for s in range(NS):
                x_sb = []
                for h in range(HT):
                    xt_h = xin.tile([P, IN_F], FP32, tag="x")
                    nc.sync.dma_start(
                        out=xt_h[:, :],
                        in_=x_d[(s * HT + h) * P : (s * HT + h + 1) * P, :],
                    )
                    x_sb.append(xt_h)

                # x^T layout: chunk k occupies cols [k*SROWS, (k+1)*SROWS),
                # half h of a chunk at col offset h*P within it.
                xt_sb = xtp.tile([P, NK * SROWS], FP32, tag="xt")
                # 64 transposes, batched 4 per PSUM tile -> 16 big copies,
                # alternating DVE/ACT. Batch i covers (k, h) pairs in xt_sb
                # column order, so each copy is one contiguous [128, 512] slab.
                for g in range(NK * HT // 4):
                    pt = tpsum.tile([P, ZC], FP32, tag="tp")
                    for i in range(4):
                        kh = 4 * g + i
                        k, h = kh // HT, kh % HT
                        nc.tensor.transpose(
                            pt[:, i * P : (i + 1) * P],
                            x_sb[h][:, k * P : (k + 1) * P],
                            ident[:, :],
                        )
                    dst = xt_sb[:, g * 4 * P : (g + 1) * 4 * P]
                    if g % 2 == 0:
                        nc.vector.tensor_copy(out=dst, in_=pt[:, :])
                    else:
                        nc.scalar.copy(out=dst, in_=pt[:, :])

                y_ps = ypsum.tile([R, SROWS], FP32, tag="y")
                for k in range(NK):
                    nc.tensor.matmul(
                        y_ps,
                        lhsT=at_sb[:, k * R : (k + 1) * R],
                        rhs=xt_sb[:, k * SROWS : (k + 1) * SROWS],
                        start=(k == 0),
                        stop=(k == NK - 1),
                    )

                # Ones-fill the whole tile (engines can't start at partition 16),
                # then overwrite rows 0:16 with y — row 16 keeps the 1.0.
                yt_sb = ytp.tile([R + 1, SROWS], FP32, tag="yt")
                nc.vector.memset(yt_sb[:, :], 1.0)
                nc.scalar.copy(out=yt_sb[0:R, :], in_=y_ps)

                for h in range(HT):
                    zrow = zrp.tile([P, OUT_F], FP32, tag="z")
                    for j in range(NJ):
                        z_ps = zpsum.tile([P, ZC], FP32, tag="zz")
                        nc.tensor.matmul(
                            z_ps,
                            lhsT=yt_sb[:, h * P : (h + 1) * P],
                            rhs=bb[:, j * ZC : (j + 1) * ZC],
                            start=True,
                            stop=True,
                        )
                        dst = zrow[:, j * ZC : (j + 1) * ZC]
                        if j % 2 == 0:
                            nc.vector.tensor_copy(out=dst, in_=z_ps)
                        else:
                            nc.scalar.copy(out=dst, in_=z_ps)
                    nc.sync.dma_start(
                        out=out_d[(s * HT + h) * P : (s * HT + h + 1) * P, :],
                        in_=zrow[:, :],
                    )

    nc.finalize()  # Bacc.finalize runs compile(): wait legalization + reg alloc
    return nc


def make_in_maps(x, lora_A, lora_B, bias):
    x2 = np.ascontiguousarray(
        np.asarray(x, dtype=np.float32).reshape(B * S, IN_F)
    )
    a = np.ascontiguousarray(np.asarray(lora_A, dtype=np.float32))
    b = np.ascontiguousarray(np.asarray(lora_B, dtype=np.float32))
    bias2 = np.ascontiguousarray(
        np.asarray(bias, dtype=np.float32).reshape(1, OUT_F)
    )
    return [
        {"x": s, "lora_A": a, "lora_B": b, "bias": bias2}
        for s in np.split(x2, N_CORES, axis=0)
    ]


def run(inputs: dict, trace: bool = False, **kw):
    global _nc_cache
    if _nc_cache is None:
        _nc_cache = build_nc()
    in_maps = make_in_maps(**inputs)
    res = run_bass_kernel_spmd(
        _nc_cache, in_maps, list(range(N_CORES)), trace=trace, **kw
    )
    out = np.concatenate(
        [res.results[i]["out"] for i in range(N_CORES)], axis=0
    ).reshape(B, S, OUT_F)
    return out, res


def kernel(**inputs) -> np.ndarray:
    out, _ = run(inputs)
    return out



# revision 2
# speedup vs baseline: 2.6379x; 2.6379x over previous
"""LoRA linear layer on 8 Trainium2 NeuronCores.

Computes out = x @ (lora_B @ lora_A * 2).T + bias for
x [4, 2048, 4096], lora_A [16, 4096], lora_B [4096, 16], bias [4096].

Strategy: pure data parallel — shard x over batch*seq (8192 rows -> 1024
rows/core), replicate the tiny LoRA weights. Exploit the rank-16 structure:
y = x @ A^T (contract 4096), z = y @ B^T + bias (contract 16+1 via the
ones-row trick), never materializing the dense 4096x4096 W.

Everything on the wire is fp16 (gate is rel_err < 2e-2; measured fp16
end-to-end error ~6e-4): x is cast AND pre-transposed on the host into the
exact SBUF layout [128 partitions = feature%128, (group, k-chunk, row)],
so the device does zero transposes and both GEMMs run at full 16-bit PE
rate with fp32 PSUM accumulation. The output travels back as fp16 and is
upcast on the host. Per-core HBM traffic: 8.4 MiB in + 8.4 MiB out.

Per-core pipeline (2 groups x 512 rows):
  1. Two 2.1 MiB DMAs per group bring in xg [128, 32*512] (k-halves split
     so GEMM1 can start after the first piece lands).
  2. GEMM1: 32 accumulating matmuls, lhsT = A2^T chunk [128,16] (x2
     pre-folded), rhs = xg chunk [128,512] -> y^T [16,512] PSUM.
  3. yt = [y^T; ones] [17,512] fp16 (memset 1.0 + cast-copy rows 0:16).
  4. GEMM2 per 128-row tile h: 8 matmuls lhsT = yt[:,h*128:+128] [17,128],
     rhs = BB chunk [17,512] (BB = [B^T; bias]) -> z [128,512] PSUM.
  5. Cast-copy z PSUM -> fp16 SBUF (alternating DVE/ACT), 1 MiB DMA out
     per row tile.
"""

import sys

import numpy as np

if "/opt/trn_rl_repo" not in sys.path:
    sys.path.insert(0, "/opt/trn_rl_repo")

import concourse.bass as bass
import concourse.mybir as mybir
from concourse import bacc
from concourse.bass_utils import run_bass_kernel_spmd
from concourse.tile import TileContext

N_CORES = 8
B, S, IN_F, OUT_F, R = 4, 2048, 4096, 4096, 16
ROWS = B * S // N_CORES  # 1024 rows per core
SCALING = 2.0  # alpha / r = 32 / 16, folded into A on the host
FP16 = mybir.dt.float16
FP32 = mybir.dt.float32
P = 128
NK = IN_F // P  # 32 contraction chunks for GEMM1
GROWS = 512  # rows per group (GEMM1 moving free dim)
NG = ROWS // GROWS  # 2 groups per core
HT = GROWS // P  # 4 row-tiles per group
ZC = 512  # GEMM2 moving chunk
NJ = OUT_F // ZC  # 8 output chunks per row tile

_nc_cache = None


def build_nc() -> bass.Bass:
    nc = bacc.Bacc()
    x_d = nc.declare_dram_parameter("xt", [P, NG * NK * GROWS], FP16, isOutput=False)
    a_d = nc.declare_dram_parameter("at", [P, NK * R], FP16, isOutput=False)
    bb_d = nc.declare_dram_parameter("bb", [R + 1, OUT_F], FP16, isOutput=False)
    out_d = nc.declare_dram_parameter("out", [ROWS, OUT_F], FP16, isOutput=True)

    with TileContext(nc) as tc:
        with (
            tc.tile_pool(name="const", bufs=1) as const,
            tc.tile_pool(name="xin", bufs=2) as xin,
            tc.tile_pool(name="zrp", bufs=3) as zrp,
            tc.tile_pool(name="ytp", bufs=2) as ytp,
            tc.tile_pool(name="ypsum", bufs=2, space="PSUM") as ypsum,
            tc.tile_pool(name="zpsum", bufs=4, space="PSUM") as zpsum,
        ):
            at_sb = const.tile([P, NK * R], FP16)
            nc.sync.dma_start(out=at_sb[:, :], in_=a_d[:, :])
            bb_sb = const.tile([R + 1, OUT_F], FP16)
            nc.sync.dma_start(out=bb_sb[:, :], in_=bb_d[:, :])

            for g in range(NG):
                # k-halves as separate DMAs so GEMM1 starts on half 0.
                xg = xin.tile([P, NK * GROWS], FP16, tag="x")
                half = NK * GROWS // 2
                for hh in range(2):
                    nc.sync.dma_start(
                        out=xg[:, hh * half : (hh + 1) * half],
                        in_=x_d[
                            :,
                            g * NK * GROWS + hh * half : g * NK * GROWS
                            + (hh + 1) * half,
                        ],
                    )

                y_ps = ypsum.tile([R, GROWS], FP32, tag="y")
                for k in range(NK):
                    nc.tensor.matmul(
                        y_ps,
                        lhsT=at_sb[:, k * R : (k + 1) * R],
                        rhs=xg[:, k * GROWS : (k + 1) * GROWS],
                        start=(k == 0),
                        stop=(k == NK - 1),
                    )

                # Ones-fill the whole tile (engines can't start at partition
                # 16), then overwrite rows 0:16 with y — row 16 keeps the 1.0.
                yt = ytp.tile([R + 1, GROWS], FP16, tag="yt")
                nc.vector.memset(yt[:, :], 1.0)
                nc.scalar.copy(out=yt[0:R, :], in_=y_ps)

                for h in range(HT):
                    zrow = zrp.tile([P, OUT_F], FP16, tag="z")
                    for j in range(NJ):
                        z_ps = zpsum.tile([P, ZC], FP32, tag="zz")
                        nc.tensor.matmul(
                            z_ps,
                            lhsT=yt[:, h * P : (h + 1) * P],
                            rhs=bb_sb[:, j * ZC : (j + 1) * ZC],
                            start=True,
                            stop=True,
                        )
                        dst = zrow[:, j * ZC : (j + 1) * ZC]
                        if j % 2 == 0:
                            nc.vector.tensor_copy(out=dst, in_=z_ps)
                        else:
                            nc.scalar.copy(out=dst, in_=z_ps)
                    nc.sync.dma_start(
                        out=out_d[(g * HT + h) * P : (g * HT + h + 1) * P, :],
                        in_=zrow[:, :],
                    )

    nc.finalize()
    return nc


def make_in_maps(x, lora_A, lora_B, bias):
    f16 = np.float16
    x2 = np.asarray(x, dtype=np.float32).reshape(B * S, IN_F).astype(f16)
    a2 = (SCALING * np.asarray(lora_A, dtype=np.float32)).astype(f16)
    # at[p, k*16+r] = 2*A[r, k*128+p]
    at = np.ascontiguousarray(
        a2.reshape(R, NK, P).transpose(2, 1, 0).reshape(P, NK * R)
    )
    bb = np.ascontiguousarray(
        np.concatenate(
            [
                np.asarray(lora_B, dtype=np.float32).T.astype(f16),
                np.asarray(bias, dtype=np.float32).astype(f16).reshape(1, OUT_F),
            ],
            axis=0,
        )
    )
    in_maps = []
    for shard in np.split(x2, N_CORES, axis=0):  # [1024, 4096] each
        # xt[p, g, k, r] = shard[g*512 + r, k*128 + p]
        xt = np.ascontiguousarray(
            shard.reshape(NG, GROWS, NK, P)
            .transpose(3, 0, 2, 1)
            .reshape(P, NG * NK * GROWS)
        )
        in_maps.append({"xt": xt, "at": at, "bb": bb})
    return in_maps


def run(inputs: dict, trace: bool = False, **kw):
    global _nc_cache
    if _nc_cache is None:
        _nc_cache = build_nc()
    in_maps = make_in_maps(**inputs)
    res = run_bass_kernel_spmd(
        _nc_cache, in_maps, list(range(N_CORES)), trace=trace, **kw
    )
    out = (
        np.concatenate([res.results[i]["out"] for i in range(N_CORES)], axis=0)
        .astype(np.float32)
        .reshape(B, S, OUT_F)
    )
    return out, res


def kernel(**inputs) -> np.ndarray:
    out, _ = run(inputs)
    return out


# revision 3
# speedup vs baseline: 2.6447x; 1.0026x over previous
"""LoRA linear layer on 8 Trainium2 NeuronCores.

Computes out = x @ (lora_B @ lora_A * 2).T + bias for
x [4, 2048, 4096], lora_A [16, 4096], lora_B [4096, 16], bias [4096].

Strategy: pure data parallel — shard x over batch*seq (8192 rows -> 1024
rows/core), replicate the tiny LoRA weights. Exploit the rank-16 structure:
y = x @ A^T (contract 4096), z = y @ B^T + bias (contract 16+1 via the
ones-row trick), never materializing the dense 4096x4096 W.

Everything on the wire is fp16 (gate is rel_err < 2e-2; measured fp16
end-to-end error ~6e-4): x is cast AND pre-transposed on the host into the
exact SBUF layout [128 partitions = feature%128, (group, k-chunk, row)],
so the device does zero transposes and both GEMMs run at full 16-bit PE
rate with fp32 PSUM accumulation. The output travels back as fp16 and is
upcast on the host. Per-core HBM traffic: 8.4 MiB in + 8.4 MiB out.

Scheduling notes (from trace analysis):
  - Concurrent DMAs queued on one HWDGE ring interleave at packet level
    and complete near-simultaneously, so inputs are split into 1 MiB
    pieces whose enqueue is paced by pool reuse (xin bufs=2: piece i+2's
    DMA waits until GEMM1 consumed piece i). Pieces then land every
    ~2.5 us and the PE starts ~14 us in instead of ~25.
  - Outputs go out via SWDGE (nc.gpsimd) — a separate descriptor path —
    so they neither queue behind inputs on the SP ring nor occupy the
    ACT queue, which is busy with PSUM->SBUF cast-copies.
  - PSUM->SBUF copies are [128, 1024] (2 PSUM banks) per instruction,
    alternating DVE/ACT, to amortize per-instruction overhead.
"""

import sys

import numpy as np

if "/opt/trn_rl_repo" not in sys.path:
    sys.path.insert(0, "/opt/trn_rl_repo")

import concourse.bass as bass
import concourse.mybir as mybir
from concourse import bacc
from concourse.bass_utils import run_bass_kernel_spmd
from concourse.tile import TileContext

N_CORES = 8
B, S, IN_F, OUT_F, R = 4, 2048, 4096, 4096, 16
ROWS = B * S // N_CORES  # 1024 rows per core
SCALING = 2.0  # alpha / r = 32 / 16, folded into A on the host
FP16 = mybir.dt.float16
FP32 = mybir.dt.float32
P = 128
NK = IN_F // P  # 32 contraction chunks for GEMM1
GROWS = 512  # rows per group (GEMM1 moving free dim)
NG = ROWS // GROWS  # 2 groups per core
HT = GROWS // P  # 4 row-tiles per group
ZC = 512  # GEMM2 moving chunk (PSUM bank = 512 fp32)
NJ = OUT_F // ZC  # 8 output chunks per row tile
PIECE_K = 8  # k-chunks per input DMA piece (1 MiB pieces)
NP = NK // PIECE_K  # 4 pieces per group

_nc_cache = None


def build_nc() -> bass.Bass:
    nc = bacc.Bacc()
    x_d = nc.declare_dram_parameter("xt", [P, NG * NK * GROWS], FP16, isOutput=False)
    a_d = nc.declare_dram_parameter("at", [P, NK * R], FP16, isOutput=False)
    bb_d = nc.declare_dram_parameter("bb", [R + 1, OUT_F], FP16, isOutput=False)
    out_d = nc.declare_dram_parameter("out", [ROWS, OUT_F], FP16, isOutput=True)

    with TileContext(nc) as tc:
        with (
            tc.tile_pool(name="const", bufs=1) as const,
            tc.tile_pool(name="xin", bufs=2) as xin,
            tc.tile_pool(name="zrp", bufs=3) as zrp,
            tc.tile_pool(name="ytp", bufs=2) as ytp,
            tc.tile_pool(name="ypsum", bufs=2, space="PSUM") as ypsum,
            tc.tile_pool(name="zpsum", bufs=3, space="PSUM") as zpsum,
        ):
            at_sb = const.tile([P, NK * R], FP16)
            nc.sync.dma_start(out=at_sb[:, :], in_=a_d[:, :])
            bb_sb = const.tile([R + 1, OUT_F], FP16)
            nc.sync.dma_start(out=bb_sb[:, :], in_=bb_d[:, :])

            PC = PIECE_K * GROWS  # columns per piece
            for g in range(NG):
                # 1 MiB input pieces; pool bufs=2 paces the enqueue so the
                # ring never interleaves more than 2 input DMAs.
                pieces = []
                for q in range(NP):
                    pt = xin.tile([P, PC], FP16, tag="x")
                    nc.sync.dma_start(
                        out=pt[:, :],
                        in_=x_d[:, (g * NP + q) * PC : (g * NP + q + 1) * PC],
                    )
                    pieces.append(pt)

                y_ps = ypsum.tile([R, GROWS], FP32, tag="y")
                for k in range(NK):
                    q, kk = k // PIECE_K, k % PIECE_K
                    nc.tensor.matmul(
                        y_ps,
                        lhsT=at_sb[:, k * R : (k + 1) * R],
                        rhs=pieces[q][:, kk * GROWS : (kk + 1) * GROWS],
                        start=(k == 0),
                        stop=(k == NK - 1),
                    )

                # Ones-fill the whole tile (engines can't start at partition
                # 16), then overwrite rows 0:16 with y — row 16 keeps the 1.0.
                yt = ytp.tile([R + 1, GROWS], FP16, tag="yt")
                nc.vector.memset(yt[:, :], 1.0)
                nc.scalar.copy(out=yt[0:R, :], in_=y_ps)

                for h in range(HT):
                    zrow = zrp.tile([P, OUT_F], FP16, tag="z")
                    for jp in range(NJ // 2):
                        z_ps = zpsum.tile([P, 2 * ZC], FP32, tag="zz")
                        for half in range(2):
                            j = 2 * jp + half
                            nc.tensor.matmul(
                                z_ps[:, half * ZC : (half + 1) * ZC],
                                lhsT=yt[:, h * P : (h + 1) * P],
                                rhs=bb_sb[:, j * ZC : (j + 1) * ZC],
                                start=True,
                                stop=True,
                            )
                        dst = zrow[:, jp * 2 * ZC : (jp + 1) * 2 * ZC]
                        if jp % 2 == 0:
                            nc.vector.tensor_copy(out=dst, in_=z_ps[:, :])
                        else:
                            nc.scalar.copy(out=dst, in_=z_ps[:, :])
                    # SWDGE path: separate descriptor ring, keeps outputs
                    # off the input ring and off the busy ACT queue.
                    nc.gpsimd.dma_start(
                        out=out_d[(g * HT + h) * P : (g * HT + h + 1) * P, :],
                        in_=zrow[:, :],
                    )

    nc.finalize()
    return nc


def make_in_maps(x, lora_A, lora_B, bias):
    f16 = np.float16
    x2 = np.asarray(x, dtype=np.float32).reshape(B * S, IN_F).astype(f16)
    a2 = (SCALING * np.asarray(lora_A, dtype=np.float32)).astype(f16)
    # at[p, k*16+r] = 2*A[r, k*128+p]
    at = np.ascontiguousarray(
        a2.reshape(R, NK, P).transpose(2, 1, 0).reshape(P, NK * R)
    )
    bb = np.ascontiguousarray(
        np.concatenate(
            [
                np.asarray(lora_B, dtype=np.float32).T.astype(f16),
                np.asarray(bias, dtype=np.float32).astype(f16).reshape(1, OUT_F),
            ],
            axis=0,
        )
    )
    in_maps = []
    for shard in np.split(x2, N_CORES, axis=0):  # [1024, 4096] each
        # xt[p, g, k, r] = shard[g*512 + r, k*128 + p]
        xt = np.ascontiguousarray(
            shard.reshape(NG, GROWS, NK, P)
            .transpose(3, 0, 2, 1)
            .reshape(P, NG * NK * GROWS)
        )
        in_maps.append({"xt": xt, "at": at, "bb": bb})
    return in_maps


def run(inputs: dict, trace: bool = False, **kw):
    global _nc_cache
    if _nc_cache is None:
        _nc_cache = build_nc()
    in_maps = make_in_maps(**inputs)
    res = run_bass_kernel_spmd(
        _nc_cache, in_maps, list(range(N_CORES)), trace=trace, **kw
    )
    out = (
        np.concatenate([res.results[i]["out"] for i in range(N_CORES)], axis=0)
        .astype(np.float32)
        .reshape(B, S, OUT_F)
    )
    return out, res


def kernel(**inputs) -> np.ndarray:
    out, _ = run(inputs)
    return out
